# revision 20
# baseline (speedup 1.0000x reference)
"""Trainium2 Bass kernel for nn_AgentEmbedding (embedding_lookup).

Contract: kernel(**inputs) takes the FULL unsharded inputs (numpy arrays,
keyed as in setup_inputs()) and returns the FULL [64, 50, 128] float32
output. Internally the batch dim B=64 is sharded 8-ways (8 per core);
the small linear weights are algebraically fused on the host (the module
is linear end-to-end) and replicated.

Per-core device program (B_local=8, T=400 tokens, E=128):
  1. 7 indirect-DMA gathers (one row id per dest partition, 512B rows)
     pull the 800 needed rows from the flat [80000,128] table.  SWDGE
     descriptor generation is engine-serial (~1.1us per instruction on
     gpsimd) and walrus only supports one offset per partition, so 7
     instructions is the floor; they are ORDERED so each output chunk's
     two source slabs arrive back to back: [k0c0, k1c0, k0c1, k1c1,
     k0c2, k1c2, tails] - chunk c closes and DMAs out while later
     gathers still run, and the last gather (32-row chunk-3 tails) has
     the smallest tail of work.
  2. While gathers run, per output chunk a PSUM accumulation group is
     opened token-major with two bf16 matmuls needing no gathered data:
     features+bias (K=12, lhsT = host-transposed feature block) and
     graph (lhsT = host-broadcast graph block).  No feature-major
     context pass, no context transposes, no final DVE add.
  3. As each gather lands: one PE transpose, one DVE/scalar copy
     (fp32->bf16) into the per-(k,chunk) lhsT tile, then two bf16
     matmuls close the chunk's PSUM group.  bf16 (tolerance 2e-2; lands
     ~2e-3) cuts PE time ~4x vs fp32r.
  4. Scalar copies each finished PSUM chunk to SBUF; sync DMAs it out.
"""

import os
import numpy as np

B, M, N, E = 64, 50, 10000, 128
NCORES = 8
BL = B // NCORES            # batches per core
T = BL * M                  # tokens per core
CHUNKS = [(0, 128), (128, 128), (256, 128), (384, 16)]  # output chunks
# gather order: (k, chunk) pairs; chunk closes right after its 2nd slab
GORDER = [(0, 0), (1, 0), (0, 1), (1, 1), (0, 2), (1, 2), (None, 3)]

_cache = {}

last_exec_time_ns = None


def _install_trace_shims():
    """antenv.axon_hooks is absent in this image; register the NTFF hook
    ourselves so run_bass_kernel_spmd(trace=True) works under axon."""
    import sys, types
    if "antenv.axon_hooks" not in sys.modules:
        mod = types.ModuleType("antenv.axon_hooks")
        store = {}
        mod.set_axon_ntff_profile_hook = lambda h: store.__setitem__("h", h)
        mod.get_axon_ntff_profile_hook = lambda: store.get("h")
        sys.modules["antenv.axon_hooks"] = mod
        try:
            from trn_agent_boot.trn_boot import _ntff_profile_via_ctypes
            mod.set_axon_ntff_profile_hook(
                _ntff_profile_via_ctypes("/opt/axon/libaxon_pjrt.so")
            )
        except Exception:
            pass
    import concourse.bass_utils as bu
    bu.upload_artifacts = lambda d: d  # zero-egress container


def _row_of(j, p):
    """Gather j, partition p -> (k, token) or None, per GORDER."""
    k, c = GORDER[j]
    if k is None:  # chunk-3 tails: both k's 16 tokens
        if p < 16:
            return (0, 384 + p)
        if p < 32:
            return (1, 384 + (p - 16))
        return None
    o, cnt = CHUNKS[c]
    return (k, o + p) if p < cnt else None


def _build_nc():
    """Build + compile the per-core Bass program (SPMD: same program on
    all 8 cores, per-core input data)."""
    import concourse.bass as bass
    import concourse.bacc as bacc
    import concourse.mybir as mybir
    import concourse.tile as tile
    from concourse.masks import make_identity

    f32 = mybir.dt.float32
    bf16 = mybir.dt.bfloat16
    i32 = mybir.dt.int32

    q2 = os.environ.get("BASS_SWDGE_Q2", "1") == "1"
    nc = bacc.Bacc("TRN2", target_bir_lowering=False,
                   dynamic_dma_scratch_size=65536,
                   num_swdge_queues=2 if q2 else 1,
                   disable_frame_to_traceback=True)
    with tile.TileContext(nc) as tc:
        with tc.tile_pool(name="dram", bufs=1, space="DRAM") as dram:
            cities = dram.tile([BL * N, E], f32, kind="ExternalInput", name="cities")
            idx2 = dram.tile([128, 8], i32, kind="ExternalInput", name="idx2")
            # all bf16 constants in one tensor/DMA: [wbig 384 | gbT 400 |
            # featw 528 (rows 0-11)]
            consts = dram.tile([128, 1312], bf16, kind="ExternalInput",
                               name="consts")
            out = dram.tile([T, E], f32, kind="ExternalOutput", name="out")
            names = dict(cities=cities.name, idx2=idx2.name,
                         consts=consts.name, out=out.name)

            with (
                tc.tile_pool(name="sb", bufs=1) as sb,
                tc.tile_pool(name="psT", bufs=2, space="PSUM") as psT,
                tc.tile_pool(name="psD", bufs=4, space="PSUM") as psD,
            ):
                idxi = sb.tile([128, 8], i32, name="idxi")
                cs = sb.tile([128, 1312], bf16, name="consts_sb")
                # layout within cs: wbig [:,0:384], gbT [:,384:784],
                # featw rows 0-12 cols 784:1312 (featT 400 | WffT 128)
                WB, GB, FW = 0, 384, 784
                ident = sb.tile([128, 128], f32, name="ident")
                ga = [sb.tile([128, E], f32, name=f"ga_{j}") for j in range(7)]
                gk = [[sb.tile([128, 128], bf16, name=f"g{k}_{c}")
                       for c in range(4)] for k in (0, 1)]
                # chunks 0/1 share one tile so one DMA writes out[0:256]
                ob01 = sb.tile([128, 2, E], f32, name="ob01")
                ob2 = sb.tile([128, E], f32, name="ob2")
                ob3 = sb.tile([128, E], f32, name="ob3")

                nc.sync.dma_start(out=idxi[:], in_=idx2[:])
                make_identity(nc, ident[:])
                nc.scalar.dma_start(out=cs[:], in_=consts[:])

                for j in range(7):
                    cnt = 32 if GORDER[j][0] is None else CHUNKS[GORDER[j][1]][1]
                    gi = nc.gpsimd.indirect_dma_start(
                        out=ga[j][:cnt, :],
                        out_offset=None,
                        in_=cities[:, :],
                        in_offset=bass.IndirectOffsetOnAxis(
                            ap=idxi[:cnt, j:j + 1], axis=0),
                    )
                    if q2 and j % 2 == 1:
                        gi.ins.queue = "qPoolDynamic1"

                # open each chunk's PSUM group token-major with the two
                # gather-independent terms: features+bias (K=12) and graph
                pg = []
                for c, (o, cnt) in enumerate(CHUNKS):
                    p = psD.tile([128, 512], f32, tag="pg", name=f"pg_{c}")
                    pg.append(p)
                    nc.tensor.matmul(out=p[:cnt, 0:E],
                                     lhsT=cs[0:12, FW + o:FW + o + cnt],
                                     rhs=cs[0:12, FW + 400:FW + 528],
                                     start=True, stop=False,
                                     skip_group_check=True)
                    nc.tensor.matmul(out=p[:cnt, 0:E],
                                     lhsT=cs[:, GB + o:GB + o + cnt],
                                     rhs=cs[:, WB + 256:WB + 384],
                                     start=False, stop=False,
                                     skip_group_check=True)

                def emit_T(j):
                    """Transpose gather j's slab and copy into its gk tile."""
                    k, c = GORDER[j]
                    rows = 32 if k is None else CHUNKS[c][1]
                    pt = psT.tile([128, 512], f32, tag="pt", name=f"pt_{j}")
                    nc.tensor.transpose(out=pt[:, 0:rows],
                                        in_=ga[j][:rows, :],
                                        identity=ident[:rows, :rows])
                    if k is None:  # tails: both k's 16 cols
                        nc.vector.tensor_copy(out=gk[0][3][:, 0:16],
                                              in_=pt[:, 0:16])
                        nc.vector.tensor_copy(out=gk[1][3][:, 0:16],
                                              in_=pt[:, 16:32])
                    elif k == 0:
                        nc.vector.tensor_copy(out=gk[0][c][:, 0:rows],
                                              in_=pt[:, 0:rows])
                    else:
                        nc.scalar.activation(
                            out=gk[1][c][:, 0:rows], in_=pt[:, 0:rows],
                            func=mybir.ActivationFunctionType.Copy)

                def emit_close(c):
                    o, cnt = CHUNKS[c]
                    nc.tensor.matmul(out=pg[c][:cnt, 0:E],
                                     lhsT=gk[0][c][:, 0:cnt],
                                     rhs=cs[:, WB:WB + 128],
                                     start=False, stop=False,
                                     skip_group_check=True)
                    nc.tensor.matmul(out=pg[c][:cnt, 0:E],
                                     lhsT=gk[1][c][:, 0:cnt],
                                     rhs=cs[:, WB + 128:WB + 256],
                                     start=False, stop=True,
                                     skip_group_check=True)
                    # PSUM -> SBUF staging, engines alternated
                    dst = (ob01[:cnt, c, :] if c < 2 else
                           ob2[:cnt, :] if c == 2 else ob3[:cnt, :])
                    if c in (0, 2):
                        nc.scalar.activation(
                            out=dst, in_=pg[c][:cnt, 0:E],
                            func=mybir.ActivationFunctionType.Copy)
                    else:
                        nc.vector.tensor_copy(out=dst, in_=pg[c][:cnt, 0:E])

                emit_T(0)
                emit_T(1)
                emit_close(0)
                emit_T(2)
                emit_T(3)
                emit_close(1)
                # one DMA covers chunks 0+1: out rows 0..255, c-major
                nc.sync.dma_start(
                    out=out[0:256, :].rearrange("(c p) e -> p c e", p=128),
                    in_=ob01[:, :, :])
                emit_T(4)
                emit_T(5)
                emit_close(2)
                nc.sync.dma_start(out=out[256:384, :], in_=ob2[:, :])
                emit_T(6)
                emit_close(3)
                nc.scalar.dma_start(out=out[384:400, :], in_=ob3[:16, :])

    nc.compile()
    return nc, names


def _host_prep(inputs):
    """Fuse the linear layers (the module has no nonlinearity) and lay out
    per-core device inputs."""
    import ml_dtypes
    bf16 = ml_dtypes.bfloat16
    f64 = np.float64
    W_a = np.asarray(inputs["W_a"], f64)
    Wa0, Wa1 = W_a[:, :E], W_a[:, E:]
    W_dp = np.asarray(inputs["W_dp"], f64)
    Wf0 = Wa1 @ W_dp[:, :E]
    Wf1 = Wa1 @ W_dp[:, E:]
    Wfc = Wa1 @ np.asarray(inputs["W_dc"], f64)
    Wfn = Wa1 @ np.asarray(inputs["W_nc"], f64)
    Wfp = Wa1 @ np.asarray(inputs["W_ps"], f64)
    Wfg = Wa0 @ np.asarray(inputs["W_g"], f64)
    b_sum = (np.asarray(inputs["b_dp"], f64) + np.asarray(inputs["b_dc"], f64)
             + np.asarray(inputs["b_nc"], f64) + np.asarray(inputs["b_ps"], f64))
    b_total = (np.asarray(inputs["b_a"], f64) + Wa1 @ b_sum
               + Wa0 @ np.asarray(inputs["b_g"], f64))

    # Wff_ext: [128, 12] fused feature weights, bias as last column
    Wff_ext = np.concatenate([Wfc, Wfn, Wfp, b_total[:, None]], axis=1)
    # wbig: [128, 384] = [Wf0^T | Wf1^T | Wfg^T] (rhs layouts, ei-major)
    wbig = np.ascontiguousarray(
        np.concatenate([Wf0.T, Wf1.T, Wfg.T], axis=1).astype(bf16))

    cities_embed = np.asarray(inputs["cities_embed"], np.float32)
    graph_embed = np.asarray(inputs["graph_embed"], np.float32)
    agent_state = np.asarray(inputs["agent_state"], np.float32)

    # packed gather grid, matching _row_of()
    k_grid = np.zeros((128, 7), np.int64)
    t_grid = np.zeros((128, 7), np.int64)
    valid = np.zeros((128, 7), bool)
    for j in range(7):
        for p in range(128):
            kt = _row_of(j, p)
            if kt is not None:
                k_grid[p, j], t_grid[p, j] = kt
                valid[p, j] = True
    bofs = (t_grid // M) * N * valid

    in_maps = []
    for core in range(NCORES):
        bsl = slice(core * BL, (core + 1) * BL)
        ag = agent_state[bsl].reshape(T, 13)
        idx2 = np.zeros((128, 8), np.int32)
        idx2[:, 0:7] = (ag[t_grid, k_grid].astype(np.int64) * valid
                        + bofs).astype(np.int32)
        # consts: [128, 1312] bf16 = [wbig 384 | gbT 400 | featw 528]
        consts = np.zeros((128, 1312), np.float32)
        consts[:, 0:384] = wbig
        gT = graph_embed[bsl, 0, :].T           # [128, 8]
        consts[:, 384:784] = np.repeat(gT, M, axis=1)
        consts[:11, 784:784 + T] = ag[:, 2:13].T
        consts[11, 784:784 + T] = 1.0
        consts[:12, 784 + 400:1312] = Wff_ext.T
        in_maps.append({
            "cities": np.ascontiguousarray(cities_embed[bsl].reshape(BL * N, E)),
            "idx2": idx2,
            "consts": consts.astype(bf16),
        })
    return in_maps


def kernel(**inputs):
    global last_exec_time_ns
    trace = os.environ.get("BASS_KERNEL_TRACE", "") == "1"
    if trace:
        _install_trace_shims()

    from concourse.bass_utils import run_bass_kernel_spmd

    if "nc" not in _cache:
        _cache["nc"], _cache["names"] = _build_nc()
    nc, names = _cache["nc"], _cache["names"]

    in_maps = []
    for m in _host_prep(inputs):
        in_maps.append({names[k]: v for k, v in m.items()})

    kwargs = {}
    if trace:
        tdir = os.environ.get("BASS_KERNEL_TRACE_DIR", "/tmp/kern_trace")
        import shutil
        shutil.rmtree(tdir, ignore_errors=True)
        os.makedirs(tdir, exist_ok=True)
        kwargs = dict(trace=True, trace_cores=list(range(NCORES)), tmpdir=tdir)
    res = run_bass_kernel_spmd(nc, in_maps, core_ids=list(range(NCORES)), **kwargs)
    last_exec_time_ns = res.exec_time_ns

    out = np.stack([res.results[i][names["out"]] for i in range(NCORES)])
    return out.reshape(B, M, E).astype(np.float32)


# revision 25
# speedup vs baseline: 1.0260x; 1.0260x over previous
"""Trainium2 Bass kernel for nn_AgentEmbedding (embedding_lookup).

Contract: kernel(**inputs) takes the FULL unsharded inputs (numpy arrays,
keyed as in setup_inputs()) and returns the FULL [64, 50, 128] float32
output. Internally the batch dim B=64 is sharded 8-ways (8 per core);
the small linear weights are algebraically fused on the host (the module
is linear end-to-end) and replicated.

Per-core device program (B_local=8, T=400 tokens, E=128):
  1. 7 indirect-DMA gathers (one row id per dest partition, 512B rows)
     pull the 800 needed rows from the flat [80000,128] table.  SWDGE
     descriptor generation is engine-serial (~1.1us per instruction on
     gpsimd) and walrus only supports one offset per partition, so 7
     instructions is the floor; they are ORDERED so each output chunk's
     two source slabs arrive back to back: [k0c0, k1c0, k0c1, k1c1,
     k0c2, k1c2, tails] - chunk c closes and DMAs out while later
     gathers still run, and the last gather (32-row chunk-3 tails) has
     the smallest tail of work.
  2. While gathers run, per output chunk a PSUM accumulation group is
     opened token-major with two bf16 matmuls needing no gathered data:
     features+bias (K=12, lhsT = host-transposed feature block) and
     graph (lhsT = host-broadcast graph block).  No feature-major
     context pass, no context transposes, no final DVE add.
  3. As each gather lands: one PE transpose, one DVE/scalar copy
     (fp32->bf16) into the per-(k,chunk) lhsT tile, then two bf16
     matmuls close the chunk's PSUM group.  bf16 (tolerance 2e-2; lands
     ~2e-3) cuts PE time ~4x vs fp32r.
  4. Scalar copies each finished PSUM chunk to SBUF; sync DMAs it out.
"""

import os
import numpy as np

B, M, N, E = 64, 50, 10000, 128
NCORES = 8
BL = B // NCORES            # batches per core
T = BL * M                  # tokens per core
CHUNKS = [(0, 128), (128, 128), (256, 128), (384, 16)]  # output chunks
# gather order: (k, chunk) pairs; chunk closes right after its 2nd slab
GORDER = [(0, 0), (1, 0), (0, 1), (1, 1), (0, 2), (1, 2), (None, 3)]

_cache = {}

last_exec_time_ns = None


def _install_trace_shims():
    """antenv.axon_hooks is absent in this image; register the NTFF hook
    ourselves so run_bass_kernel_spmd(trace=True) works under axon."""
    import sys, types
    if "antenv.axon_hooks" not in sys.modules:
        mod = types.ModuleType("antenv.axon_hooks")
        store = {}
        mod.set_axon_ntff_profile_hook = lambda h: store.__setitem__("h", h)
        mod.get_axon_ntff_profile_hook = lambda: store.get("h")
        sys.modules["antenv.axon_hooks"] = mod
        try:
            from trn_agent_boot.trn_boot import _ntff_profile_via_ctypes
            mod.set_axon_ntff_profile_hook(
                _ntff_profile_via_ctypes("/opt/axon/libaxon_pjrt.so")
            )
        except Exception:
            pass
    import concourse.bass_utils as bu
    bu.upload_artifacts = lambda d: d  # zero-egress container


def _row_of(j, p):
    """Gather j, partition p -> (k, token) or None, per GORDER."""
    k, c = GORDER[j]
    if k is None:  # chunk-3 tails: both k's 16 tokens
        if p < 16:
            return (0, 384 + p)
        if p < 32:
            return (1, 384 + (p - 16))
        return None
    o, cnt = CHUNKS[c]
    return (k, o + p) if p < cnt else None


def _build_nc():
    """Build + compile the per-core Bass program (SPMD: same program on
    all 8 cores, per-core input data)."""
    import concourse.bass as bass
    import concourse.bacc as bacc
    import concourse.mybir as mybir
    import concourse.tile as tile
    from concourse.masks import make_identity

    f32 = mybir.dt.float32
    bf16 = mybir.dt.bfloat16
    i32 = mybir.dt.int32

    nq = int(os.environ.get("BASS_SWDGE_NQ", "4"))
    nc = bacc.Bacc("TRN2", target_bir_lowering=False,
                   dynamic_dma_scratch_size=65536,
                   num_swdge_queues=nq,
                   disable_frame_to_traceback=True)
    with tile.TileContext(nc) as tc:
        with tc.tile_pool(name="dram", bufs=1, space="DRAM") as dram:
            cities = dram.tile([BL * N, E], f32, kind="ExternalInput", name="cities")
            idx2 = dram.tile([128, 8], i32, kind="ExternalInput", name="idx2")
            # all bf16 constants in one tensor/DMA: [wbig 384 | gbT 400 |
            # featw 528 (rows 0-11)]
            consts = dram.tile([128, 1312], bf16, kind="ExternalInput",
                               name="consts")
            out = dram.tile([T, E], f32, kind="ExternalOutput", name="out")
            names = dict(cities=cities.name, idx2=idx2.name,
                         consts=consts.name, out=out.name)

            with (
                tc.tile_pool(name="sb", bufs=1) as sb,
                tc.tile_pool(name="psT", bufs=2, space="PSUM") as psT,
                tc.tile_pool(name="psD", bufs=4, space="PSUM") as psD,
            ):
                idxi = sb.tile([128, 8], i32, name="idxi")
                cs = sb.tile([128, 1312], bf16, name="consts_sb")
                # layout within cs: wbig [:,0:384], gbT [:,384:784],
                # featw rows 0-12 cols 784:1312 (featT 400 | WffT 128)
                WB, GB, FW = 0, 384, 784
                ident = sb.tile([128, 128], f32, name="ident")
                ga = [sb.tile([128, E], f32, name=f"ga_{j}") for j in range(7)]
                gk = [[sb.tile([128, 128], bf16, name=f"g{k}_{c}")
                       for c in range(4)] for k in (0, 1)]
                gt3 = sb.tile([128, 32], bf16, name="gt3")
                # chunks 0/1 share one tile so one DMA writes out[0:256]
                ob01 = sb.tile([128, 2, E], f32, name="ob01")
                ob2 = sb.tile([128, E], f32, name="ob2")
                ob3 = sb.tile([128, E], f32, name="ob3")

                nc.sync.dma_start(out=idxi[:], in_=idx2[:])
                make_identity(nc, ident[:])
                nc.scalar.dma_start(out=cs[:], in_=consts[:])

                for j in range(7):
                    cnt = 32 if GORDER[j][0] is None else CHUNKS[GORDER[j][1]][1]
                    gi = nc.gpsimd.indirect_dma_start(
                        out=ga[j][:cnt, :],
                        out_offset=None,
                        in_=cities[:, :],
                        in_offset=bass.IndirectOffsetOnAxis(
                            ap=idxi[:cnt, j:j + 1], axis=0),
                    )
                    if j % nq:
                        gi.ins.queue = f"qPoolDynamic{j % nq}"

                # open each chunk's PSUM group token-major with the two
                # gather-independent terms: features+bias (K=12) and graph
                pg = []
                for c, (o, cnt) in enumerate(CHUNKS):
                    p = psD.tile([128, 512], f32, tag="pg", name=f"pg_{c}")
                    pg.append(p)
                    nc.tensor.matmul(out=p[:cnt, 0:E],
                                     lhsT=cs[0:12, FW + o:FW + o + cnt],
                                     rhs=cs[0:12, FW + 400:FW + 528],
                                     start=True, stop=False,
                                     skip_group_check=True)
                    nc.tensor.matmul(out=p[:cnt, 0:E],
                                     lhsT=cs[:, GB + o:GB + o + cnt],
                                     rhs=cs[:, WB + 256:WB + 384],
                                     start=False, stop=False,
                                     skip_group_check=True)

                def emit_T(j):
                    """Transpose gather j's slab and copy into its gk tile."""
                    k, c = GORDER[j]
                    rows = 32 if k is None else CHUNKS[c][1]
                    pt = psT.tile([128, 512], f32, tag="pt", name=f"pt_{j}")
                    nc.tensor.transpose(out=pt[:, 0:rows],
                                        in_=ga[j][:rows, :],
                                        identity=ident[:rows, :rows])
                    if k is None:  # tails: both k's 16 cols, one cast
                        nc.vector.tensor_copy(out=gt3[:, 0:32],
                                              in_=pt[:, 0:32])
                    elif k == 0:
                        nc.vector.tensor_copy(out=gk[0][c][:, 0:rows],
                                              in_=pt[:, 0:rows])
                    else:
                        nc.scalar.activation(
                            out=gk[1][c][:, 0:rows], in_=pt[:, 0:rows],
                            func=mybir.ActivationFunctionType.Copy)

                def emit_close(c):
                    o, cnt = CHUNKS[c]
                    l0 = gt3[:, 0:16] if c == 3 else gk[0][c][:, 0:cnt]
                    l1 = gt3[:, 16:32] if c == 3 else gk[1][c][:, 0:cnt]
                    nc.tensor.matmul(out=pg[c][:cnt, 0:E],
                                     lhsT=l0,
                                     rhs=cs[:, WB:WB + 128],
                                     start=False, stop=False,
                                     skip_group_check=True)
                    nc.tensor.matmul(out=pg[c][:cnt, 0:E],
                                     lhsT=l1,
                                     rhs=cs[:, WB + 128:WB + 256],
                                     start=False, stop=True,
                                     skip_group_check=True)
                    # PSUM -> SBUF staging, engines alternated
                    dst = (ob01[:cnt, c, :] if c < 2 else
                           ob2[:cnt, :] if c == 2 else ob3[:cnt, :])
                    if c in (0, 2):
                        nc.scalar.activation(
                            out=dst, in_=pg[c][:cnt, 0:E],
                            func=mybir.ActivationFunctionType.Copy)
                    else:
                        nc.vector.tensor_copy(out=dst, in_=pg[c][:cnt, 0:E])

                emit_T(0)
                emit_T(1)
                emit_close(0)
                emit_T(2)
                emit_T(3)
                emit_close(1)
                # one DMA covers chunks 0+1: out rows 0..255, c-major
                nc.sync.dma_start(
                    out=out[0:256, :].rearrange("(c p) e -> p c e", p=128),
                    in_=ob01[:, :, :])
                emit_T(4)
                emit_T(5)
                emit_close(2)
                nc.sync.dma_start(out=out[256:384, :], in_=ob2[:, :])
                emit_T(6)
                emit_close(3)
                nc.scalar.dma_start(out=out[384:400, :], in_=ob3[:16, :])

    nc.compile()
    return nc, names


def _host_prep(inputs):
    """Fuse the linear layers (the module has no nonlinearity) and lay out
    per-core device inputs."""
    import ml_dtypes
    bf16 = ml_dtypes.bfloat16
    f64 = np.float64
    W_a = np.asarray(inputs["W_a"], f64)
    Wa0, Wa1 = W_a[:, :E], W_a[:, E:]
    W_dp = np.asarray(inputs["W_dp"], f64)
    Wf0 = Wa1 @ W_dp[:, :E]
    Wf1 = Wa1 @ W_dp[:, E:]
    Wfc = Wa1 @ np.asarray(inputs["W_dc"], f64)
    Wfn = Wa1 @ np.asarray(inputs["W_nc"], f64)
    Wfp = Wa1 @ np.asarray(inputs["W_ps"], f64)
    Wfg = Wa0 @ np.asarray(inputs["W_g"], f64)
    b_sum = (np.asarray(inputs["b_dp"], f64) + np.asarray(inputs["b_dc"], f64)
             + np.asarray(inputs["b_nc"], f64) + np.asarray(inputs["b_ps"], f64))
    b_total = (np.asarray(inputs["b_a"], f64) + Wa1 @ b_sum
               + Wa0 @ np.asarray(inputs["b_g"], f64))

    # Wff_ext: [128, 12] fused feature weights, bias as last column
    Wff_ext = np.concatenate([Wfc, Wfn, Wfp, b_total[:, None]], axis=1)
    # wbig: [128, 384] = [Wf0^T | Wf1^T | Wfg^T] (rhs layouts, ei-major)
    wbig = np.ascontiguousarray(
        np.concatenate([Wf0.T, Wf1.T, Wfg.T], axis=1).astype(bf16))

    cities_embed = np.asarray(inputs["cities_embed"], np.float32)
    graph_embed = np.asarray(inputs["graph_embed"], np.float32)
    agent_state = np.asarray(inputs["agent_state"], np.float32)

    # packed gather grid, matching _row_of()
    k_grid = np.zeros((128, 7), np.int64)
    t_grid = np.zeros((128, 7), np.int64)
    valid = np.zeros((128, 7), bool)
    for j in range(7):
        for p in range(128):
            kt = _row_of(j, p)
            if kt is not None:
                k_grid[p, j], t_grid[p, j] = kt
                valid[p, j] = True
    bofs = (t_grid // M) * N * valid

    in_maps = []
    for core in range(NCORES):
        bsl = slice(core * BL, (core + 1) * BL)
        ag = agent_state[bsl].reshape(T, 13)
        idx2 = np.zeros((128, 8), np.int32)
        idx2[:, 0:7] = (ag[t_grid, k_grid].astype(np.int64) * valid
                        + bofs).astype(np.int32)
        # consts: [128, 1312] bf16 = [wbig 384 | gbT 400 | featw 528]
        consts = np.zeros((128, 1312), np.float32)
        consts[:, 0:384] = wbig
        gT = graph_embed[bsl, 0, :].T           # [128, 8]
        consts[:, 384:784] = np.repeat(gT, M, axis=1)
        consts[:11, 784:784 + T] = ag[:, 2:13].T
        consts[11, 784:784 + T] = 1.0
        consts[:12, 784 + 400:1312] = Wff_ext.T
        in_maps.append({
            "cities": np.ascontiguousarray(cities_embed[bsl].reshape(BL * N, E)),
            "idx2": idx2,
            "consts": consts.astype(bf16),
        })
    return in_maps


def kernel(**inputs):
    global last_exec_time_ns
    trace = os.environ.get("BASS_KERNEL_TRACE", "") == "1"
    if trace:
        _install_trace_shims()

    from concourse.bass_utils import run_bass_kernel_spmd

    if "nc" not in _cache:
        _cache["nc"], _cache["names"] = _build_nc()
    nc, names = _cache["nc"], _cache["names"]

    in_maps = []
    for m in _host_prep(inputs):
        in_maps.append({names[k]: v for k, v in m.items()})

    kwargs = {}
    if trace:
        tdir = os.environ.get("BASS_KERNEL_TRACE_DIR", "/tmp/kern_trace")
        import shutil
        shutil.rmtree(tdir, ignore_errors=True)
        os.makedirs(tdir, exist_ok=True)
        kwargs = dict(trace=True, trace_cores=list(range(NCORES)), tmpdir=tdir)
    res = run_bass_kernel_spmd(nc, in_maps, core_ids=list(range(NCORES)), **kwargs)
    last_exec_time_ns = res.exec_time_ns

    out = np.stack([res.results[i][names["out"]] for i in range(NCORES)])
    return out.reshape(B, M, E).astype(np.float32)
